# revision 2
# baseline (speedup 1.0000x reference)
"""GATConv Trainium kernel v5 (single-core SPMD program) + host prep.

Deltas vs v4 (all validated by microbench):
  - One-hot planes (sel for the scatter, selT for the a_dst gather) are
    HOST-BUILT fp8e4 tensors (exact 0/1), loaded per block. Kills the
    on-device dst broadcast + both DVE one-hot compares + scalar copy.
  - fp8 lhsT x bf16 rhs matmuls (validated exact; fp8 FWL = faster LDW).
  - acc matmul split in two: [Gs|ea] from the rhs tile (132 cols) and the
    plain-h part STRAIGHT FROM the gather stage (strided rhs) -> no h copy.
  - Self-loop edges leave the gather path entirely: one dense strided DMA
    per block + identity one-hot (a_dst = a_blk directly, no par matmul).
  - x is uploaded bf16 (phase-1 load is the matmul operand directly).
  - a_src accumulated into pa PSUM via identity-weights matmul.
"""

import numpy as np

import concourse.bass as bass
import concourse.bacc as bacc
import concourse.mybir as mybir
import concourse.tile as tile
from concourse import library_config

DT = mybir.dt
ALU = mybir.AluOpType
ACTF = mybir.ActivationFunctionType

F = 128    # feature dim (in == out)
NH = 4     # heads
HD = 32    # head dim
GE = 256   # h_ext row width in bf16 elems (512B)
HEC = 132  # used cols: h(128) | a_src(4)
RC = 260   # acc cols: Gs(128) | ea(4) | h(128)
RHC = 132  # rhs tile cols: Gs(128) | ea(4)
GU = 8     # tiles per gather/compute unit
CB = 8     # phase-1 chunks per batch

N_NODES = 50000
N_CORES = 8
DEV_N = N_NODES // N_CORES          # 6250
NBLK = (DEV_N + 127) // 128         # 49
NCH = (N_NODES + 127) // 128        # 391 chunks
V2 = NCH * 128                      # 50048 table rows
HALF = 32768


def build_gat_nc(T_LO, T_HI, leaky=0.2):
    T = T_LO + T_HI
    NT = NBLK * T

    nc = bacc.Bacc(num_swdge_queues=4)
    xT = nc.declare_dram_parameter("xT", [F, N_NODES], DT.bfloat16,
                                   isOutput=False)
    Wnat = nc.declare_dram_parameter("Wnat", [F, F], DT.float32,
                                     isOutput=False)
    Wt = nc.declare_dram_parameter("Wt", [F, F], DT.float32, isOutput=False)
    Aatt = nc.declare_dram_parameter("Aatt", [F, 2 * NH], DT.float32,
                                     isOutput=False)
    gidx = nc.declare_dram_parameter("gidx", [128, NT * 8], DT.int16,
                                     isOutput=False)
    selP = nc.declare_dram_parameter("selP", [128, NT * 128], DT.float8e4,
                                     isOutput=False)
    selTP = nc.declare_dram_parameter("selTP", [128, NT * 128], DT.float8e4,
                                      isOutput=False)
    out = nc.declare_dram_parameter("out", [DEV_N, F], DT.float32,
                                    isOutput=True)

    h_ext = nc.dram_tensor("h_ext", [V2, GE], DT.bfloat16)
    a_dev = nc.dram_tensor("a_dev", [NBLK * 128, NH], DT.bfloat16)

    with tile.TileContext(nc) as tc:
        with (
            tc.tile_pool(name="const", bufs=1) as const,
            tc.tile_pool(name="p1", bufs=3) as p1,
            tc.tile_pool(name="psA", bufs=2, space="PSUM") as psA,
            tc.tile_pool(name="p2", bufs=2) as p2,
            tc.tile_pool(name="pu", bufs=3) as pu,
            tc.tile_pool(name="pst", bufs=8) as pst,
            tc.tile_pool(name="psB", bufs=2, space="PSUM") as psB,
            tc.tile_pool(name="psP", bufs=2, space="PSUM") as psP,
        ):
            nc.gpsimd.load_library(library_config.mlp)

            # ---- constants ----
            wnat_t = const.tile([128, F], DT.float32)
            aatt_t = const.tile([128, 2 * NH], DT.float32)
            wt_t = const.tile([128, F], DT.float32)
            wextb = const.tile([128, 256], DT.bfloat16)
            iota16 = const.tile([128, 128], DT.int16)
            iota_c = const.tile([128, 1], DT.float32)
            idn = const.tile([128, 128], DT.float8e4)
            nc.sync.dma_start(out=wnat_t[:], in_=Wnat[:, :])
            nc.sync.dma_start(out=aatt_t[:], in_=Aatt[:, :])
            nc.sync.dma_start(out=wt_t[:], in_=Wt[:, :])
            nc.gpsimd.iota(iota16[:], pattern=[[1, 128]], base=0,
                           channel_multiplier=0,
                           allow_small_or_imprecise_dtypes=True)
            nc.gpsimd.iota(iota_c[:], pattern=[[0, 1]], base=0,
                           channel_multiplier=1,
                           allow_small_or_imprecise_dtypes=True)
            nc.vector.memset(wextb[:], 0.0)
            nc.vector.tensor_scalar(out=idn[:], in0=iota16[:],
                                    scalar1=iota_c[:, 0:1], scalar2=None,
                                    op0=ALU.is_equal)
            vps = psA.tile([128, 256], DT.float32, tag="hps")
            nc.tensor.matmul(out=vps[:, 0:2 * NH], lhsT=wnat_t[:],
                             rhs=aatt_t[:], start=True, stop=True)
            nc.vector.tensor_copy(out=wextb[:, 0:F], in_=wt_t[:])
            # cols 128:132 = W.T@A_src (table), 132:136 = W.T@A_dst (a_dev)
            nc.vector.tensor_copy(out=wextb[:, F:F + 2 * NH],
                                  in_=vps[:, 0:2 * NH])

            # ---- phase 1 ----
            for cb in range(0, NCH, CB):
                nb = min(CB, NCH - cb)
                c0n = cb * 128
                nn = min(N_NODES - c0n, nb * 128)
                xcb = p1.tile([128, CB * 128], DT.bfloat16, tag="xcb")
                nc.scalar.dma_start(out=xcb[:, :nn], in_=xT[:, c0n:c0n + nn])
                hrow = p1.tile([128, CB * HEC], DT.bfloat16, tag="hrow")
                arow = p1.tile([128, CB * NH], DT.bfloat16, tag="arow")
                for k in range(nb):
                    m = min(128, N_NODES - (c0n + k * 128))
                    hps = psA.tile([128, 256], DT.float32, tag="hps")
                    nc.tensor.matmul(out=hps[:m, :],
                                     lhsT=xcb[:, k * 128:k * 128 + m],
                                     rhs=wextb[:], start=True, stop=True)
                    nc.scalar.copy(out=hrow[:m, k * HEC:(k + 1) * HEC],
                                   in_=hps[:m, 0:HEC])
                    if cb + k < NBLK:
                        nc.scalar.copy(out=arow[:m, k * NH:(k + 1) * NH],
                                       in_=hps[:m, HEC:HEC + NH])
                kfull = (min(N_NODES, c0n + nb * 128) - c0n) // 128
                if kfull > 0:
                    nc.sync.dma_start(
                        out=bass.AP(h_ext[:, :].tensor, cb * GE,
                                    [[NCH * GE, 128], [GE, kfull], [1, HEC]]),
                        in_=hrow[:].rearrange("p (k f) -> p k f", f=HEC)[
                            :, 0:kfull, :])
                if kfull < nb:
                    mp = N_NODES - (c0n + kfull * 128)
                    nc.sync.dma_start(
                        out=bass.AP(h_ext[:, :].tensor, (cb + kfull) * GE,
                                    [[NCH * GE, mp], [1, HEC]]),
                        in_=hrow[:mp, kfull * HEC:(kfull + 1) * HEC])
                ka = max(0, min(nb, NBLK - cb))
                if ka > 0:
                    nc.sync.dma_start(
                        out=bass.AP(a_dev[:, :].tensor, cb * 128 * NH,
                                    [[NH, 128], [128 * NH, ka], [1, NH]]),
                        in_=arow[:].rearrange("p (k e) -> p k e", e=NH)[
                            :, 0:ka, :])

            # ---- phase 2 ----
            gcnt = 0
            for b in range(NBLK):
                rows = min(128, DEV_N - b * 128)
                gi = p2.tile([128, T * 8], DT.int16, tag="gi")
                nc.sync.dma_start(out=gi[:],
                                  in_=gidx[:, b * T * 8:(b + 1) * T * 8])
                sp = p2.tile([128, T * 128], DT.float8e4, tag="sp")
                nc.sync.dma_start(
                    out=sp[:], in_=selP[:, b * T * 128:(b + 1) * T * 128])
                spr = sp[:].rearrange("p (t m) -> p t m", m=128)
                stp = p2.tile([128, T * 128], DT.float8e4, tag="stp")
                nc.sync.dma_start(
                    out=stp[:], in_=selTP[:, b * T * 128:(b + 1) * T * 128])
                stpr = stp[:].rearrange("p (t e) -> p t e", e=128)
                a_blk = p2.tile([128, NH], DT.bfloat16, tag="a_blk")
                nc.sync.dma_start(out=a_blk[:],
                                  in_=a_dev[b * 128:(b + 1) * 128, 0:NH])
                acc = psB.tile([128, RC], DT.float32, tag="acc")

                # -- self-loop unit (dense rows, identity one-hot) --
                sf = p2.tile([128, HEC], DT.bfloat16, tag="sf")
                nc.sync.dma_start(
                    out=sf[:],
                    in_=bass.AP(h_ext[:, :].tensor, b * GE,
                                [[NCH * GE, 128], [1, HEC]]))
                scrS = p2.tile([128, NH], DT.float32, tag="scrS")
                nc.vector.tensor_tensor(out=scrS[:], in0=sf[:, F:F + NH],
                                        in1=a_blk[:], op=ALU.add)
                scrS2 = p2.tile([128, NH], DT.float32, tag="scrS2")
                nc.vector.tensor_scalar_mul(out=scrS2[:], in0=scrS[:],
                                            scalar1=leaky)
                nc.vector.tensor_tensor(out=scrS[:], in0=scrS[:],
                                        in1=scrS2[:], op=ALU.max)
                rhS = p2.tile([128, RC], DT.bfloat16, tag="rhS")
                nc.scalar.activation(out=rhS[:, F:F + NH], in_=scrS[:],
                                     func=ACTF.Exp)
                nc.vector.tensor_tensor(
                    out=rhS[:, 0:F].rearrange("p (h e) -> p h e", e=HD),
                    in0=sf[:, 0:F].rearrange("p (h e) -> p h e", e=HD),
                    in1=rhS[:, F:F + NH][:, :, None].to_broadcast(
                        [128, NH, HD]),
                    op=ALU.mult)
                nc.scalar.copy(out=rhS[:, RHC:RC], in_=sf[:, 0:F])
                # single start=True matmul covering the full acc region
                nc.tensor.matmul(out=acc[:], lhsT=idn[:], rhs=rhS[:],
                                 start=True, stop=False)

                units = []
                for t0, Ts, roff in ((0, T_LO, 0), (T_LO, T_HI, HALF)):
                    for u in range(0, Ts, GU):
                        units.append((t0 + u, min(GU, Ts - u), roff))

                stages = []
                for t0, Tu, roff in units:
                    st = pst.tile([128, GU * GE], DT.bfloat16, tag="stage")
                    sr = st[:].rearrange("p (t f) -> p t f", f=GE)
                    nc.gpsimd.dma_gather(
                        out_ap=sr[:, 0:Tu, :],
                        in_ap=bass.AP(h_ext[:, :].tensor, roff * GE,
                                      [[GE, V2 - roff], [1, GE]]),
                        idxs_ap=gi[:, t0 * 8:(t0 + Tu) * 8],
                        num_idxs=Tu * 128, num_idxs_reg=Tu * 128,
                        elem_size=GE, single_packet=False,
                        queue_num=gcnt % 4)
                    gcnt += 1
                    stages.append(sr)

                for ui, (t0, Tu, roff) in enumerate(units):
                    sr = stages[ui]
                    # alpha = a_src (identity mm) + a_dst (one-hot mm)
                    pa = psP.tile([128, GU * NH], DT.float32, tag="pa")
                    nc.tensor.matmul(
                        out=pa[:, 0:Tu * NH],
                        lhsT=idn[:],
                        rhs=sr[:, 0:Tu, F:F + NH],
                        start=True, stop=False)
                    for j in range(Tu):
                        nc.tensor.matmul(
                            out=pa[:, j * NH:(j + 1) * NH],
                            lhsT=stpr[:, t0 + j, :],
                            rhs=a_blk[:], start=False, stop=True)
                    # leaky relu on DVE; exp on scalar
                    scr2 = pu.tile([128, GU * NH], DT.float32, tag="scr2")
                    nc.vector.tensor_scalar_mul(out=scr2[:, 0:Tu * NH],
                                                in0=pa[:, 0:Tu * NH],
                                                scalar1=leaky)
                    scr = pu.tile([128, GU * NH], DT.float32, tag="scr")
                    nc.vector.tensor_tensor(out=scr[:, 0:Tu * NH],
                                            in0=pa[:, 0:Tu * NH],
                                            in1=scr2[:, 0:Tu * NH],
                                            op=ALU.max)
                    rhs = pu.tile([128, GU * RHC], DT.bfloat16, tag="rhs")
                    rr = rhs[:].rearrange("p (t c) -> p t c", c=RHC)
                    nc.scalar.activation(
                        out=rr[:, 0:Tu, F:F + NH],
                        in_=scr[:, 0:Tu * NH].rearrange(
                            "p (t h) -> p t h", h=NH),
                        func=ACTF.Exp)
                    # Gs = h * ea (per-head broadcast)
                    nc.vector.tensor_tensor(
                        out=rr[:, 0:Tu, 0:F].rearrange(
                            "p t (h e) -> p t h e", e=HD),
                        in0=sr[:, 0:Tu, 0:F].rearrange(
                            "p t (h e) -> p t h e", e=HD),
                        in1=rr[:, 0:Tu, F:F + NH][:, :, :, None].to_broadcast(
                            [128, Tu, NH, HD]),
                        op=ALU.mult)
                    # accumulate: [Gs|ea] from rhs tile, h straight from stage
                    for j in range(Tu):
                        nc.tensor.matmul(
                            out=acc[:, 0:RHC],
                            lhsT=spr[:, t0 + j, :],
                            rhs=rr[:, j, :],
                            start=False,
                            stop=(ui == len(units) - 1 and j == Tu - 1))
                        nc.tensor.matmul(
                            out=acc[:, RHC:RC],
                            lhsT=spr[:, t0 + j, :],
                            rhs=sr[:, j, 0:F],
                            start=False,
                            stop=(ui == len(units) - 1 and j == Tu - 1))

                # ---- evac: out = P / s + Q ----
                sden = p2.tile([128, NH], DT.float32, tag="sden")
                nc.vector.tensor_scalar_max(out=sden[:],
                                            in0=acc[:, F:F + NH],
                                            scalar1=1e-30)
                rs = p2.tile([128, NH], DT.float32, tag="rs")
                nc.vector.reciprocal(out=rs[:], in_=sden[:])
                ot = p2.tile([128, F], DT.float32, tag="ot")
                otr = ot[:].rearrange("p (h e) -> p h e", e=HD)
                nc.vector.tensor_tensor(
                    out=otr,
                    in0=acc[:, 0:F].rearrange("p (h e) -> p h e", e=HD),
                    in1=rs[:][:, :, None].to_broadcast([128, NH, HD]),
                    op=ALU.mult)
                nc.vector.tensor_tensor(
                    out=otr, in0=otr,
                    in1=acc[:, RHC:RC].rearrange("p (h e) -> p h e", e=HD),
                    op=ALU.add)
                nc.sync.dma_start(out=out[b * 128:b * 128 + rows, :],
                                  in_=ot[:rows, :])

    return nc


def route_edges(edge_index):
    """Host edge routing (self-loops excluded -> dense self unit)."""
    import ml_dtypes
    N = N_NODES
    src = np.asarray(edge_index[0]).astype(np.int64)
    dst = np.asarray(edge_index[1]).astype(np.int64)
    keep = src != dst          # true self-edges in the data stay in sections
    # NOTE: the +self-loop the reference APPENDS is handled densely; data
    # edges that happen to have src==dst must stay in the general path.
    core = dst // DEV_N

    per_core_raw = []
    T_LO = T_HI = 1
    for d in range(N_CORES):
        m = core == d
        s_rot = (src[m] - d * DEV_N) % N
        rr = (s_rot % 128) * NCH + s_rot // 128
        dloc = (dst[m] - d * DEV_N).astype(np.int64)
        blk = dloc // 128
        mloc = (dloc % 128).astype(np.int16)
        sec = (rr >= HALF).astype(np.int64)
        key = blk * 2 + sec
        cnts = np.bincount(key, minlength=NBLK * 2)
        T_LO = max(T_LO, int(-(-cnts[0::2].max() // 128)))
        T_HI = max(T_HI, int(-(-cnts[1::2].max() // 128)))
        per_core_raw.append((rr, blk, mloc, sec, key))

    T = T_LO + T_HI
    NT = NBLK * T
    per_core = []
    for d in range(N_CORES):
        rr, blk, mloc, sec, key = per_core_raw[d]
        order = np.argsort(key, kind="stable")
        cnts = np.bincount(key, minlength=NBLK * 2)
        starts = np.concatenate([[0], np.cumsum(cnts)[:-1]])
        within = np.empty(len(key), dtype=np.int64)
        within[order] = np.arange(len(key)) - starts[key[order]]
        tile0 = np.where(sec == 0, 0, T_LO)
        base = blk * T + tile0
        g = np.zeros((16, NT * 8), dtype=np.int16)
        g[within % 16, base * 8 + within // 16] = np.where(
            sec == 0, rr, rr - HALF).astype(np.int16)
        lane = (within % 128).astype(np.int64)
        tcol = base + within // 128
        sP = np.zeros((128, NT * 128), dtype=ml_dtypes.float8_e4m3fn)
        sP[lane, tcol * 128 + mloc] = 1.0
        sTP = np.zeros((128, NT * 128), dtype=ml_dtypes.float8_e4m3fn)
        sTP[mloc.astype(np.int64), tcol * 128 + lane] = 1.0
        per_core.append({
            "gidx": np.tile(g, (8, 1)),
            "selP": sP,
            "selTP": sTP,
        })
    return T_LO, T_HI, per_core


def host_prep(x, edge_index, W, att_src, att_dst):
    import ml_dtypes
    xTf = np.asarray(x).T.astype(ml_dtypes.bfloat16)
    Wnat = np.ascontiguousarray(np.asarray(W).astype(np.float32))
    Wt = np.ascontiguousarray(Wnat.T)
    A = np.zeros((F, 2 * NH), dtype=np.float32)
    for h in range(NH):
        A[h * HD:(h + 1) * HD, h] = np.asarray(att_src)[0, h]
        A[h * HD:(h + 1) * HD, NH + h] = np.asarray(att_dst)[0, h]
    T_LO, T_HI, per_core = route_edges(edge_index)
    in_maps = []
    for d in range(N_CORES):
        xr = np.roll(xTf, -d * DEV_N, axis=1)
        in_maps.append(dict(per_core[d], xT=np.ascontiguousarray(xr),
                            Wnat=Wnat, Wt=Wt, Aatt=A))
    return T_LO, T_HI, in_maps


def _run(inputs, trace=False):
    import time
    from concourse.bass_utils import run_bass_kernel_spmd

    x = np.asarray(inputs["x"], dtype=np.float32)
    edge_index = np.asarray(inputs["edge_index"])
    W = np.asarray(inputs["W"], dtype=np.float32)
    att_src = np.asarray(inputs["att_src"], dtype=np.float32)
    att_dst = np.asarray(inputs["att_dst"], dtype=np.float32)
    assert x.shape[0] == N_NODES

    t0 = time.time()
    T_LO, T_HI, in_maps = host_prep(x, edge_index, W, att_src, att_dst)
    t1 = time.time()
    nc = build_gat_nc(T_LO, T_HI)
    nc.compile()
    t2 = time.time()
    res = run_bass_kernel_spmd(nc, in_maps, list(range(N_CORES)), trace=trace)
    t3 = time.time()
    print(f"kernel: host_prep {t1-t0:.1f}s build+compile {t2-t1:.1f}s "
          f"run {t3-t2:.1f}s T_LO={T_LO} T_HI={T_HI}")
    out = np.concatenate([res.results[d]["out"] for d in range(N_CORES)],
                         axis=0).astype(np.float32)
    return out, res.exec_time_ns


def kernel(**inputs) -> np.ndarray:
    return _run(inputs, trace=False)[0]
